# revision 8
# baseline (speedup 1.0000x reference)
"""Causal self-attention (B=1, T=4096, D=1024, H=16, dh=64) on 8 trn2 NeuronCores.

Sharding: tensor-parallel over heads — each core owns 2 of the 16 heads.
Per core: QKV projection (transposed activation layout), RoPE, causal
flash-style attention with transposed score tiles St[k,q] (so the AV matmul
needs no P transposes), softmax denominator via an appended ones-column in V,
out-projection against this core's W_out column slice -> partial output
[T, D].  Host sums the 8 partials.

All matmuls run with float32 data bitcast to float32r (full PE rate at
moving-dim >= 256, fp32 accumulation in PSUM).
"""

import sys

sys.path.insert(0, "/opt/trn_rl_repo")

import numpy as np

import concourse.bass as bass
import concourse.tile as tile
from concourse import bacc, mybir
from concourse.bass_utils import run_bass_kernel_spmd

T = 4096
D = 1024
H = 16
DH = 64
NC = 8
HL = H // NC  # heads per core (2)
DL = HL * DH  # local feature width (128)

F32 = mybir.dt.float32
F32R = mybir.dt.float32r


def _r(ap):
    return ap


def build_nc():
    nc = bacc.Bacc(
        "TRN2", target_bir_lowering=False, debug=False, num_devices=NC
    )

    # ---- DRAM I/O -------------------------------------------------------
    xT_d = nc.dram_tensor("xT", [D, T], F32R, kind="ExternalInput").ap()
    wqkvT_d = nc.dram_tensor("wqkvT", [D, 3 * DL], F32R, kind="ExternalInput").ap()
    woutT_d = nc.dram_tensor("woutT", [DL, D], F32R, kind="ExternalInput").ap()
    cos2_d = nc.dram_tensor("cos2", [DL, T], F32R, kind="ExternalInput").ap()
    sin2_d = nc.dram_tensor("sin2", [DL, T], F32R, kind="ExternalInput").ap()
    p128_d = nc.dram_tensor("p128", [DL, DL], F32R, kind="ExternalInput").ap()
    ident_d = nc.dram_tensor("ident", [128, 128], F32, kind="ExternalInput").ap()
    e2_d = nc.dram_tensor("e2", [HL, 128], F32R, kind="ExternalInput").ap()
    # 4 diagonal-block masks [128, 512]: mask_j[k, q] = 1 iff q >= j*128 + k
    dmask_d = nc.dram_tensor("dmask", [128, 4 * 512], F32R, kind="ExternalInput").ap()
    out_d = nc.dram_tensor("outp", [T, D], F32, kind="ExternalOutput").ap()

    # internal DRAM scratch for the softmax-sum partition shuffle
    sums_d = nc.dram_tensor("sums_scratch", [HL, T], F32R).ap()
    recip_d = nc.dram_tensor("recip_scratch", [HL, T], F32R).ap()

    NCH = 8  # T-chunks of 512 for the QKV projection
    CW = 512  # chunk width
    VBLK = 130  # v-nat block layout: [v_h0(64) | 1 | v_h1(64) | 1]

    with tile.TileContext(nc) as tc:
        with tc.tile_pool(name="consts", bufs=1) as cpool, \
             tc.tile_pool(name="persist", bufs=1) as ppool:
            # ---- constants ---------------------------------------------
            wt = []
            for d in range(8):
                w = cpool.tile([128, 3 * DL], F32R, tag=f"wt{d}")
                nc.sync.dma_start(out=w[:], in_=wqkvT_d[d * 128:(d + 1) * 128, :])
                wt.append(w)
            woutT = cpool.tile([DL, D], F32R, tag="woutT")
            nc.sync.dma_start(out=woutT[:], in_=woutT_d)
            cos2 = cpool.tile([DL, T], F32R, tag="cos2")
            nc.sync.dma_start(out=cos2[:], in_=cos2_d)
            sin2 = cpool.tile([DL, T], F32R, tag="sin2")
            nc.sync.dma_start(out=sin2[:], in_=sin2_d)
            p128 = cpool.tile([DL, DL], F32R, tag="p128")
            nc.sync.dma_start(out=p128[:], in_=p128_d)
            ident = cpool.tile([128, 128], F32, tag="ident")
            nc.sync.dma_start(out=ident[:], in_=ident_d)
            e2 = cpool.tile([HL, 128], F32R, tag="e2")
            nc.sync.dma_start(out=e2[:], in_=e2_d)
            dmask = cpool.tile([128, 4 * 512], F32R, tag="dmask")
            nc.sync.dma_start(out=dmask[:], in_=dmask_d)

            # ---- persistent activations --------------------------------
            qT = ppool.tile([DL, T], F32R, tag="qT")
            kT = ppool.tile([DL, T], F32R, tag="kT")
            vnat = ppool.tile([128, (T // 128) * VBLK], F32R, tag="vnat")
            attnT = ppool.tile([DL, T], F32R, tag="attnT")

            # ones columns of the v-nat layout (cols 64 and 129 of each block):
            # memset an f32 staging tile, DVE-copy (casts+rounds) into the
            # strided f32r columns.
            ones_sb = cpool.tile([128, T // 128], F32, tag="ones_sb")
            nc.gpsimd.memset(ones_sb[:], 1.0)
            vone = vnat[:].rearrange("p (b c) -> p b c", c=VBLK)
            nc.vector.tensor_copy(vone[:, :, 64:65], ones_sb[:].rearrange("p (b c) -> p b c", c=1))
            nc.vector.tensor_copy(vone[:, :, 129:130], ones_sb[:].rearrange("p (b c) -> p b c", c=1))

            # ================= Phase A: QKV + RoPE ======================
            with tc.tile_pool(name="xp", bufs=2) as xpool, \
                 tc.tile_pool(name="tmpa", bufs=3) as tpool, \
                 tc.tile_pool(name="psA", bufs=2, space="PSUM") as psA:
                for c in range(NCH):
                    s = c * CW
                    xt = xpool.tile([128, 8 * CW], F32R, tag="xchunk")
                    for d in range(8):
                        nc.sync.dma_start(
                            out=xt[:, d * CW:(d + 1) * CW],
                            in_=xT_d[d * 128:(d + 1) * 128, s:s + CW],
                        )

                    def xs(d):
                        return xt[:, d * CW:(d + 1) * CW]

                    # qT / kT with RoPE
                    for idx, dst in ((0, qT), (1, kT)):
                        pp = psA.tile([128, CW], F32, tag="qkvps")
                        for d in range(8):
                            nc.tensor.matmul(
                                pp[:],
                                lhsT=_r(wt[d][:, idx * DL:(idx + 1) * DL]),
                                rhs=_r(xs(d)),
                                start=(d == 0),
                                stop=(d == 7),
                            )
                        praw = tpool.tile([128, CW], F32R, tag="praw")
                        if idx == 0:
                            nc.vector.tensor_copy(praw[:], pp[:])
                        else:
                            nc.scalar.copy(praw[:], pp[:])
                        rot = psA.tile([128, CW], F32, tag="rotps")
                        nc.tensor.matmul(
                            rot[:], lhsT=_r(p128[:]), rhs=_r(praw[:]),
                            start=True, stop=True,
                        )
                        dstv = dst[:, s:s + CW]
                        nc.vector.tensor_mul(dstv, praw[:], cos2[:, s:s + CW])
                        rtmp = tpool.tile([128, CW], F32R, tag="rtmp")
                        nc.vector.tensor_mul(rtmp[:], rot[:], sin2[:, s:s + CW])
                        nc.gpsimd.tensor_add(dstv, dstv, rtmp[:])

                    # v: compute vT then PE-transpose to natural layout
                    vp = psA.tile([128, CW], F32, tag="qkvps")
                    for d in range(8):
                        nc.tensor.matmul(
                            vp[:], lhsT=_r(wt[d][:, 2 * DL:3 * DL]),
                            rhs=_r(xs(d)), start=(d == 0), stop=(d == 7),
                        )
                    vtmp = tpool.tile([128, CW], F32, tag="vtmp")
                    nc.scalar.copy(vtmp[:], vp[:])
                    for b in range(CW // 128):
                        kb = (s // 128) + b
                        tp = psA.tile([128, 128], F32, tag="vtps")
                        nc.tensor.transpose(
                            tp[:], vtmp[:, b * 128:(b + 1) * 128], ident[:]
                        )
                        o = kb * VBLK
                        nc.vector.tensor_copy(vnat[:, o:o + 64], tp[:, 0:64])
                        nc.vector.tensor_copy(vnat[:, o + 65:o + 129], tp[:, 64:128])

            # ================= Phase B: attention =======================
            with tc.tile_pool(name="ptp", bufs=3) as ptpool, \
                 tc.tile_pool(name="evp", bufs=3) as evpool, \
                 tc.tile_pool(name="psB", bufs=2, space="PSUM") as psB:
                for h in range(HL):
                    hs = h * DH
                    for qc in range(8):
                        q0 = qc * 512
                        kmax = 4 * (qc + 1)
                        at = psB.tile([DH + 1, 512], F32, tag="atps")
                        for g in range((kmax + 1) // 2):
                            kbs = [kb for kb in (2 * g, 2 * g + 1) if kb < kmax]
                            st = psB.tile([128, 1024], F32, tag="stps")
                            for i, kb in enumerate(kbs):
                                nc.tensor.matmul(
                                    st[:, i * 512:(i + 1) * 512],
                                    lhsT=_r(kT[hs:hs + DH, kb * 128:(kb + 1) * 128]),
                                    rhs=_r(qT[hs:hs + DH, q0:q0 + 512]),
                                    start=True, stop=True,
                                )
                            pt = ptpool.tile([128, 1024], F32R, tag="pt")
                            w = 512 * len(kbs)
                            nc.scalar.activation(
                                pt[:, 0:w], st[:, 0:w],
                                mybir.ActivationFunctionType.Exp,
                                scale=0.125,
                            )
                            for i, kb in enumerate(kbs):
                                j = kb - 4 * qc
                                if j >= 0:
                                    nc.vector.tensor_mul(
                                        pt[:, i * 512:(i + 1) * 512],
                                        pt[:, i * 512:(i + 1) * 512],
                                        dmask[:, j * 512:(j + 1) * 512],
                                    )
                            for i, kb in enumerate(kbs):
                                o = kb * VBLK + h * 65
                                nc.tensor.matmul(
                                    at[:],
                                    lhsT=_r(vnat[:, o:o + 65]),
                                    rhs=_r(pt[:, i * 512:(i + 1) * 512]),
                                    start=(kb == 0), stop=(kb == kmax - 1),
                                    skip_group_check=True,
                                )
                        # evacuate: attn rows + sums row
                        ev = evpool.tile([DH + 1, 512], F32R, tag="ev")
                        nc.vector.tensor_copy(ev[:], at[:])
                        nc.sync.dma_start(
                            out=attnT[hs:hs + DH, q0:q0 + 512], in_=ev[0:DH, :]
                        )
                        nc.sync.dma_start(
                            out=sums_d[h, q0:q0 + 512], in_=ev[DH:DH + 1, :]
                        )

            # ================= normalize ================================
            with tc.tile_pool(name="nrm", bufs=1) as npool, \
                 tc.tile_pool(name="psN", bufs=2, space="PSUM") as psN:
                scom = npool.tile([128, HL * 32], F32R, tag="scom")
                # sums[h, t] with t = qc*512 + p*4 + f  ->  scom[p, h*32+qc*4+f]
                nc.sync.dma_start(
                    out=scom[:].rearrange("p (h qc f) -> p h qc f", h=HL, qc=8),
                    in_=sums_d.rearrange("h (qc p f) -> p h qc f", qc=8, p=128),
                )
                rcom = npool.tile([128, HL * 32], F32R, tag="rcom")
                with nc.allow_low_precision(reason="fp32r rounding of softmax recip"):
                    nc.vector.reciprocal(rcom[:], scom[:])
                nc.sync.dma_start(
                    out=recip_d.rearrange("h (qc p f) -> p h qc f", qc=8, p=128),
                    in_=rcom[:].rearrange("p (h qc f) -> p h qc f", h=HL, qc=8),
                )
                rrows = npool.tile([HL, T], F32R, tag="rrows")
                nc.sync.dma_start(out=rrows[:], in_=recip_d)
                for qc in range(8):
                    rb = psN.tile([128, 512], F32, tag="rbps")
                    nc.tensor.matmul(
                        rb[:], lhsT=_r(e2[:]),
                        rhs=_r(rrows[:, qc * 512:(qc + 1) * 512]),
                        start=True, stop=True,
                    )
                    nc.vector.tensor_mul(
                        attnT[:, qc * 512:(qc + 1) * 512],
                        attnT[:, qc * 512:(qc + 1) * 512],
                        rb[:],
                    )

            # ================= Phase C: out projection ==================
            with tc.tile_pool(name="op", bufs=3) as opool, \
                 tc.tile_pool(name="psC", bufs=3, space="PSUM") as psC:
                for tb in range(T // 128):
                    osb = opool.tile([128, D], F32, tag="osb")
                    for ec in range(2):
                        op = psC.tile([128, 512], F32, tag="ops")
                        nc.tensor.matmul(
                            op[:],
                            lhsT=_r(attnT[:, tb * 128:(tb + 1) * 128]),
                            rhs=_r(woutT[:, ec * 512:(ec + 1) * 512]),
                            start=True, stop=True,
                        )
                        nc.vector.tensor_copy(osb[:, ec * 512:(ec + 1) * 512], op[:])
                    nc.sync.dma_start(
                        out=out_d[tb * 128:(tb + 1) * 128, :], in_=osb[:]
                    )

    nc.compile()
    return nc


def _round_f32r(a):
    """Round fp32 array to the fp32r format (12-bit mantissa, RNE-ish)."""
    b = np.ascontiguousarray(a, np.float32).view(np.uint32)
    b = ((b + 0x800) & np.uint32(0xFFFFF000)).astype(np.uint32)
    return b.view(np.float32)


def _host_constants():
    inv_freq = 1.0 / (10000.0 ** (np.arange(0, DH, 2, dtype=np.float64) / DH))
    t = np.arange(T, dtype=np.float64)
    freqs = np.outer(t, inv_freq)  # [T, 32]
    emb = np.concatenate([freqs, freqs], axis=-1)  # [T, 64]
    cos = np.cos(emb).astype(np.float32).T  # [64, T]
    sin = np.sin(emb).astype(np.float32).T  # [64, T]
    sinS = sin.copy()
    sinS[0:DH // 2] *= -1.0  # fold rotate_half's negation into the table
    cos2 = np.ascontiguousarray(np.tile(cos, (HL, 1)))  # [128, T]
    sin2 = np.ascontiguousarray(np.tile(sinS, (HL, 1)))

    # swap-halves permutation (per 64-row head block), symmetric
    p1 = np.zeros((DH, DH), np.float32)
    half = DH // 2
    p1[np.arange(half), np.arange(half) + half] = 1.0
    p1[np.arange(half) + half, np.arange(half)] = 1.0
    p128 = np.block([
        [p1, np.zeros((DH, DH), np.float32)],
        [np.zeros((DH, DH), np.float32), p1],
    ]).astype(np.float32)

    ident = np.eye(128, dtype=np.float32)

    e2 = np.zeros((HL, 128), np.float32)
    for h in range(HL):
        e2[h, h * DH:(h + 1) * DH] = 1.0

    # diag masks [128, 4*512]: mask_j[k, q] = 1 iff q >= j*128 + k
    dmask = np.zeros((128, 4, 512), np.float32)
    kk = np.arange(128)[:, None]
    qq = np.arange(512)[None, :]
    for j in range(4):
        dmask[:, j, :] = (qq >= j * 128 + kk).astype(np.float32)
    dmask = np.ascontiguousarray(dmask.reshape(128, 4 * 512))

    return cos2, sin2, p128, ident, e2, dmask


_NC_CACHE = None


def _get_nc():
    global _NC_CACHE
    if _NC_CACHE is None:
        _NC_CACHE = build_nc()
    return _NC_CACHE


def _in_maps(x, W_qkv, W_out):
    x2 = np.asarray(x, np.float32).reshape(T, D)
    W_qkv = np.asarray(W_qkv, np.float32)
    W_out = np.asarray(W_out, np.float32)
    xT = _round_f32r(np.ascontiguousarray(x2.T))
    cos2, sin2, p128, ident, e2, dmask = _host_constants()
    cos2 = _round_f32r(cos2)
    sin2 = _round_f32r(sin2)

    Wq, Wk, Wv = W_qkv[0:D], W_qkv[D:2 * D], W_qkv[2 * D:3 * D]
    in_maps = []
    for c in range(NC):
        h0, h1 = HL * c, HL * c + 1
        rows = []
        for Wp in (Wq, Wk, Wv):
            rows.append(Wp[h0 * DH:(h0 + 1) * DH])
            rows.append(Wp[h1 * DH:(h1 + 1) * DH])
        wqkvT = _round_f32r(np.concatenate(rows, axis=0).T)  # [D, 384]
        cols = np.r_[h0 * DH:(h0 + 1) * DH, h1 * DH:(h1 + 1) * DH]
        woutT = _round_f32r(W_out[:, cols].T)  # [128, D]
        in_maps.append({
            "xT": xT, "wqkvT": wqkvT, "woutT": woutT,
            "cos2": cos2, "sin2": sin2, "p128": p128,
            "ident": ident, "e2": e2, "dmask": dmask,
        })
    return in_maps


def _run(x, W_qkv, W_out, **spmd_kwargs):
    nc = _get_nc()
    res = run_bass_kernel_spmd(
        nc, _in_maps(x, W_qkv, W_out), core_ids=list(range(NC)), **spmd_kwargs
    )
    out = res.results[0]["outp"].astype(np.float64)
    for c in range(1, NC):
        out += res.results[c]["outp"]
    return out.astype(np.float32).reshape(1, T, D), res


def kernel(x, W_qkv, W_out):
    out, _ = _run(x, W_qkv, W_out)
    return out
